# revision 11
# baseline (speedup 1.0000x reference)
"""CBOW hierarchical-softmax loss kernel for 8x TRN2 NeuronCores.

Strategy: data-parallel over the batch dim (8192 examples per core), both
embedding tables replicated per core. Partition p of a core owns examples
p*64 .. p*64+63; each of 32 iterations processes 2 examples per partition:
  - indirect-DMA row gathers from in_embed (2*10 rows/partition) and
    node_embed (2*18 rows/partition), 512 B per row
  - DVE pairwise-tree sum over the C=10 context rows
  - fused tensor_tensor_reduce (mult + add-reduce over E=128) per (ex, d)
  - sign/scale fold: t2 = t_raw * (2*code-1)/C
  - ACT sigmoid, then Ln(x + eps) with accum_out summing over the D=18
    path positions -> per-example loss column
Final negate + single store of the [128, 64] loss tile per core.
"""

import numpy as np

B, C, D = 65536, 10, 18
V, NN, E = 100000, 99999, 128
EPS = 1e-9
P = 128
N_CORES = 8
B_CORE = B // N_CORES  # 8192
EX = 2  # examples per partition per iteration

_cache = {}


def _build(b_core=B_CORE, ex=EX):
    import concourse.bass as bass
    import concourse.mybir as mybir
    import concourse.tile as tile
    from concourse import bacc

    j = b_core // P           # examples per partition
    iters = j // ex
    assert j % ex == 0

    f32 = mybir.dt.float32
    i32 = mybir.dt.int32
    AF = mybir.ActivationFunctionType
    OP = mybir.AluOpType

    nc = bacc.Bacc(
        "TRN2",
        target_bir_lowering=False,
        debug=False,
        enable_asserts=False,
    )

    ctx_d = nc.dram_tensor("ctx_idx", [b_core, C], i32, kind="ExternalInput")
    path_d = nc.dram_tensor("path_idx", [b_core, D], i32, kind="ExternalInput")
    codes_d = nc.dram_tensor("codes", [b_core, D], i32, kind="ExternalInput")
    emb_d = nc.dram_tensor("in_embed", [V, E], f32, kind="ExternalInput")
    nemb_d = nc.dram_tensor("node_embed", [NN, E], f32, kind="ExternalInput")
    loss_d = nc.dram_tensor("loss", [b_core], f32, kind="ExternalOutput")

    from contextlib import ExitStack

    with tile.TileContext(nc) as tc, ExitStack() as ctx:
        res_pool = ctx.enter_context(tc.tile_pool(name="resident", bufs=1))
        ct_pool = ctx.enter_context(tc.tile_pool(name="ct", bufs=2))
        ut_pool = ctx.enter_context(tc.tile_pool(name="ut", bufs=2))
        small_pool = ctx.enter_context(tc.tile_pool(name="small", bufs=2))

        # resident index / code tiles: partition p holds its 64 examples
        ctxi = res_pool.tile([P, j * C], i32)
        nc.sync.dma_start(ctxi[:], ctx_d.ap().rearrange("(p j) c -> p (j c)", p=P))
        pathi = res_pool.tile([P, j * D], i32)
        nc.sync.dma_start(pathi[:], path_d.ap().rearrange("(p j) c -> p (j c)", p=P))
        codesr = res_pool.tile([P, j * D], i32)
        nc.sync.dma_start(codesr[:], codes_d.ap().rearrange("(p j) c -> p (j c)", p=P))

        lacc = res_pool.tile([P, j], f32)        # +sum of logs, negated at end
        eps_t = res_pool.tile([P, 1], f32)       # Ln bias (+eps)
        nc.vector.memset(eps_t[:], EPS)

        for k in range(iters):
            # ---- gathers: one indirect DMA per slot (128 rows each) ----
            ct = ct_pool.tile([P, ex * C * E], f32)
            for sl in range(ex * C):
                nc.gpsimd.indirect_dma_start(
                    out=ct[:, sl * E:(sl + 1) * E],
                    out_offset=None,
                    in_=emb_d.ap(),
                    in_offset=bass.IndirectOffsetOnAxis(
                        ap=ctxi[:, k * ex * C + sl:k * ex * C + sl + 1], axis=0
                    ),
                )
            ut = ut_pool.tile([P, ex * D * E], f32)
            for sl in range(ex * D):
                nc.gpsimd.indirect_dma_start(
                    out=ut[:, sl * E:(sl + 1) * E],
                    out_offset=None,
                    in_=nemb_d.ap(),
                    in_offset=bass.IndirectOffsetOnAxis(
                        ap=pathi[:, k * ex * D + sl:k * ex * D + sl + 1], axis=0
                    ),
                )

            # ---- context sum over c (tree, in-place in ct) ----
            # view [p][s][c][e]
            ct4 = ct[:].rearrange("p (s c e) -> p s c e", s=ex, c=C, e=E)
            nc.vector.tensor_tensor(
                out=ct4[:, :, 0:5, :], in0=ct4[:, :, 0:5, :],
                in1=ct4[:, :, 5:10, :], op=OP.add,
            )
            nc.vector.tensor_tensor(
                out=ct4[:, :, 0:2, :], in0=ct4[:, :, 0:2, :],
                in1=ct4[:, :, 2:4, :], op=OP.add,
            )
            nc.vector.tensor_tensor(
                out=ct4[:, :, 0:1, :], in0=ct4[:, :, 0:1, :],
                in1=ct4[:, :, 1:2, :], op=OP.add,
            )
            nc.vector.tensor_tensor(
                out=ct4[:, :, 0:1, :], in0=ct4[:, :, 0:1, :],
                in1=ct4[:, :, 4:5, :], op=OP.add,
            )

            # ---- dot products over e: w = u * v (broadcast over d), then
            # segmented reduce over e; logits t = reduce / C ----
            w4 = ut_pool.tile([P, ex * D * E], f32, tag="w4")
            nc.vector.tensor_tensor(
                out=w4[:].rearrange("p (s d e) -> p s d e", s=ex, d=D, e=E),
                in0=ut[:].rearrange("p (s d e) -> p s d e", s=ex, d=D, e=E),
                in1=ct[:].rearrange("p (s c e) -> p s c e", s=ex, c=C, e=E)[
                    :, :, 0:1, :].to_broadcast([P, ex, D, E]),
                op=OP.mult,
            )
            traw = small_pool.tile([P, ex * D], f32)
            nc.vector.tensor_reduce(
                out=traw[:],
                in_=w4[:].rearrange("p (s d e) -> p (s d) e", s=ex, d=D, e=E),
                axis=mybir.AxisListType.X,
                op=OP.add,
            )
            # ---- replicate reference numerics: s = 1/(1+exp(-t)) in fp32,
            # p = s (code==1) else 1-s.  1-s == (1+u)-1 bit-exactly in the
            # tail (incl. the snap-to-zero), where u = exp(-t), t = traw/C
            # (the 1/C mean scale is folded into the Exp scale). ----
            ue = small_pool.tile([P, ex * D], f32)
            nc.scalar.activation(out=ue[:], in_=traw[:], func=AF.Exp, scale=-1.0 / C)
            w = small_pool.tile([P, ex * D], f32)
            nc.vector.tensor_scalar_add(w[:], ue[:], 1.0)
            r = small_pool.tile([P, ex * D], f32)
            nc.vector.reciprocal(r[:], w[:])
            pm1 = small_pool.tile([P, ex * D], f32)
            nc.vector.tensor_scalar(
                out=pm1[:], in0=r[:], scalar1=-1.0, scalar2=1.0,
                op0=OP.mult, op1=OP.add,
            )
            pp = small_pool.tile([P, ex * D], f32)
            nc.vector.select(
                pp[:], codesr[:, k * ex * D:(k + 1) * ex * D], r[:], pm1[:]
            )

            # ---- log(p + eps), sum over d ----
            lg = small_pool.tile([P, ex * D], f32)
            for s in range(ex):
                nc.scalar.activation(
                    out=lg[:, s * D:(s + 1) * D],
                    in_=pp[:, s * D:(s + 1) * D],
                    func=AF.Ln,
                    bias=eps_t[:, 0:1],
                    accum_out=lacc[:, k * ex + s: k * ex + s + 1],
                )

        lout = res_pool.tile([P, j], f32)
        nc.vector.tensor_scalar_mul(lout[:], lacc[:], -1.0)
        nc.sync.dma_start(loss_d.ap().rearrange("(p j) -> p j", p=P), lout[:])

    nc.compile()
    return nc


def _get_nc(b_core=B_CORE, ex=EX):
    key = (b_core, ex)
    if key not in _cache:
        _cache[key] = _build(b_core, ex)
    return _cache[key]


def kernel(context_idxs, path_nodes, codes, in_embed, node_embed):
    from concourse.bass_utils import run_bass_kernel_spmd

    context_idxs = np.ascontiguousarray(np.asarray(context_idxs, dtype=np.int32))
    path_nodes = np.ascontiguousarray(np.asarray(path_nodes, dtype=np.int32))
    codes = np.ascontiguousarray(np.asarray(codes, dtype=np.int32))
    in_embed = np.ascontiguousarray(np.asarray(in_embed, dtype=np.float32))
    node_embed = np.ascontiguousarray(np.asarray(node_embed, dtype=np.float32))

    nc = _get_nc()
    in_maps = []
    for m in range(N_CORES):
        sl = slice(m * B_CORE, (m + 1) * B_CORE)
        in_maps.append(
            {
                "ctx_idx": context_idxs[sl],
                "path_idx": path_nodes[sl],
                "codes": codes[sl],
                "in_embed": in_embed,
                "node_embed": node_embed,
            }
        )
    res = run_bass_kernel_spmd(nc, in_maps, core_ids=list(range(N_CORES)))
    return np.concatenate([r["loss"] for r in res.results]).astype(np.float32)
